# revision 7
# baseline (speedup 1.0000x reference)
"""DETR3D cross-attention Bass kernel for 8 trn2 NeuronCores.

Strategy: queries sharded 8 ways ((batch, query-block) grid: 2 batches x 4
query slices of 225). Each core holds its batch's feature pyramid in HBM as
pixel-major tables ([row=pixel, 256ch], per (cam, level), with 1 lead + 2
trail pad rows) and sparse-gathers only the bilinear-corner pixel pairs it
needs via the gpsimd dma_gather custom op (int16 wrapped indices). The
(cam, level, corner) weighted reduction runs as fused scalar_tensor_tensor
ops on DVE plus diag-matmul accumulation on the tensor engine; projection,
attention weights, output projection and the positional-embedding MLP all
run on-device. No collectives: each core produces a disjoint output slice.
"""

import numpy as np

import concourse.bacc as bacc
import concourse.bass as bass
import concourse.mybir as mybir
import concourse.tile as tile
from concourse.bass_utils import run_bass_kernel_spmd

F32 = mybir.dt.float32
I16 = mybir.dt.int16
I32 = mybir.dt.int32
ALU = mybir.AluOpType
ACTF = mybir.ActivationFunctionType

B, Q, C, N, L = 2, 900, 256, 6, 3
LV = [(116, 200), (58, 100), (29, 50)]            # (H, W) per level
ROWS_L = [h * w + 3 for h, w in LV]               # 1 lead + HW + 2 trail pad rows
LBASE = [0, ROWS_L[0], ROWS_L[0] + ROWS_L[1]]     # level base within a cam's table
CAM_ROWS = sum(ROWS_L)                            # 30459 (< int16 max)
TOTAL_ROWS = N * CAM_ROWS
IMG_W, IMG_H = 1600.0, 928.0
EPS = 1e-5
PC = (-51.2, -51.2, -5.0, 51.2, 51.2, 3.0)

NQ_CORE = Q // 4                                  # 225 queries per core
CHUNKS = [(0, 128), (128, 97)]                    # (row0, qc)
NT = N * L                                        # 18 gather tables
WALL_S = 32                                       # idx cols per table (512 idx / 16)
NUM_IDX = 512                                     # 4 groups of 128 (chunk x ycorner)

# tuning knobs
N_QUEUES = 4
G_BUFS = 12
PE_MOD = 3          # term t goes to PE when t % PE_MOD != PE_MOD-1, else DVE

_CACHE = {}


def _build():
    nc = bacc.Bacc("TRN2", target_bir_lowering=False, debug=False,
                   num_swdge_queues=N_QUEUES)

    din = {}
    def dram(name, shape, dtype=F32, kind="ExternalInput"):
        din[name] = nc.dram_tensor(name, shape, dtype, kind=kind)
        return din[name]

    featT = dram("featT", [TOTAL_ROWS, C])
    xq = dram("xq", [NQ_CORE, C])
    xqp = dram("xqp", [NQ_CORE, C])
    refp = dram("refp", [NQ_CORE, 3])
    l2iT = dram("l2iT", [4, 4 * N])
    Wa = dram("Wa", [C, 18]); ba = dram("ba", [1, 18])
    Wo = dram("Wo", [C, C]); bo = dram("bo", [1, C])
    Wp1 = dram("Wp1", [3, C]); bp1 = dram("bp1", [1, C])
    Wp2 = dram("Wp2", [C, C]); bp2 = dram("bp2", [1, C])
    ident_in = dram("ident", [128, 128])
    sel16_in = dram("sel16", [16, 128])
    pcsb_in = dram("pcsb", [4, 2])
    out_d = dram("out", [NQ_CORE, C], kind="ExternalOutput")

    v = nc.vector
    s = nc.scalar

    with tile.TileContext(nc) as tc:
        with (
            tc.tile_pool(name="cst", bufs=1) as cst,
            tc.tile_pool(name="wk", bufs=2) as wk,
            tc.tile_pool(name="gp", bufs=G_BUFS) as gp,
            tc.tile_pool(name="dg", bufs=4) as dg,
            tc.tile_pool(name="pT", bufs=2, space="PSUM") as pT,
            tc.tile_pool(name="pH", bufs=1, space="PSUM") as pH,
            tc.tile_pool(name="pA", bufs=2, space="PSUM") as pA,
            tc.tile_pool(name="pO", bufs=1, space="PSUM") as pO,
            tc.tile_pool(name="pW", bufs=1, space="PSUM") as pW,
        ):
            # ---- constants ----
            ident = cst.tile([128, 128], F32, name="ident")
            nc.sync.dma_start(ident[:], ident_in.ap())
            sel16 = cst.tile([16, 128], F32, name="sel16")
            nc.sync.dma_start(sel16[:], sel16_in.ap())
            l2i_t = cst.tile([4, 4 * N], F32, name="l2i_t")
            nc.sync.dma_start(l2i_t[:], l2iT.ap())
            wa_t = [cst.tile([128, 18], F32, name=f"wa{k}") for k in range(2)]
            for k in range(2):
                nc.sync.dma_start(wa_t[k][:], Wa.ap()[k * 128:(k + 1) * 128, :])
            wo_t = [cst.tile([128, C], F32, name=f"wo{k}") for k in range(2)]
            for k in range(2):
                nc.sync.dma_start(wo_t[k][:], Wo.ap()[k * 128:(k + 1) * 128, :])
            wp2_t = [cst.tile([128, C], F32, name=f"wp2{k}") for k in range(2)]
            for k in range(2):
                nc.sync.dma_start(wp2_t[k][:], Wp2.ap()[k * 128:(k + 1) * 128, :])
            wp1_t = cst.tile([3, C], F32, name="wp1_t")
            nc.sync.dma_start(wp1_t[:], Wp1.ap())
            ba_t = cst.tile([1, 18], F32, name="ba_t")
            nc.sync.dma_start(ba_t[:], ba.ap())
            bo_t = cst.tile([1, C], F32, name="bo_t")
            nc.sync.dma_start(bo_t[:], bo.ap())
            bp1_t = cst.tile([1, C], F32, name="bp1_t")
            nc.sync.dma_start(bp1_t[:], bp1.ap())
            bp2_t = cst.tile([1, C], F32, name="bp2_t")
            nc.sync.dma_start(bp2_t[:], bp2.ap())
            ones1 = cst.tile([1, 128], F32, name="ones1")
            v.memset(ones1[:], 1.0)
            pcsb = cst.tile([4, 2], F32, name="pcsb")
            nc.sync.dma_start(pcsb[:], pcsb_in.ap())

            wallf = cst.tile([16, NT * WALL_S], F32, name="wallf")
            v.memset(wallf[:], 0.0)
            wall = cst.tile([128, NT * WALL_S], I16, name="wall")

            cd = []  # per-chunk tiles kept across phases

            # ---- phase A: projection + gather indices (per chunk) ----
            for ci, (row0, qc) in enumerate(CHUNKS):
                d = {}
                ref4 = wk.tile([qc, 4], F32, name="ref4")
                nc.sync.dma_start(ref4[:, 0:3], refp.ap()[row0:row0 + qc, :])
                v.memset(ref4[:, 3:4], 1.0)
                rT_ps = pT.tile([4, qc], F32, name="rT_ps", tag="tp", space="PSUM")
                nc.tensor.transpose(rT_ps[:], ref4[:], ident[:qc, :qc])
                refT4 = wk.tile([4, qc], F32, name="refT4")
                s.copy(refT4[:], rT_ps[:])
                d["refT"] = refT4[0:3, :]

                homT = wk.tile([4, qc], F32, name="homT")
                s.activation(homT[:], refT4[:], ACTF.Identity,
                             bias=pcsb[:, 1:2], scale=pcsb[:, 0:1])

                cam_ps = pT.tile([qc, 4 * N], F32, name="cam_ps", tag="tp", space="PSUM")
                nc.tensor.matmul(cam_ps[:], lhsT=homT[:, :], rhs=l2i_t[:, :],
                                 start=True, stop=True)
                cam = wk.tile([qc, 4 * N], F32, name="cam")
                v.tensor_copy(cam[:], cam_ps[:])

                z = cam[:, 2::4]
                zc = wk.tile([qc, N], F32, name="zc")
                v.tensor_scalar(zc[:], z, EPS, None, op0=ALU.max)
                rz = wk.tile([qc, N], F32, name="rz")
                v.reciprocal(rz[:], zc[:])
                px = wk.tile([qc, N], F32, name="px")
                v.tensor_tensor(out=px[:], in0=cam[:, 0::4], in1=rz[:], op=ALU.mult)
                py = wk.tile([qc, N], F32, name="py")
                v.tensor_tensor(out=py[:], in0=cam[:, 1::4], in1=rz[:], op=ALU.mult)
                d["z"], d["px"], d["py"] = z, px, py

                # level pixel coords, floors, fracs, valids, clipped indices
                W18 = lambda nm: wk.tile([qc, N * L], F32, name=nm)
                xln, yln = W18("xln"), W18("yln")
                x0t, y0t = W18("x0t"), W18("y0t")
                wx1, wy1 = W18("wx1"), W18("wy1")
                wx0, wy0 = W18("wx0"), W18("wy0")
                vx0, vx1 = W18("vx0"), W18("vx1")
                vy0, vy1 = W18("vy0"), W18("vy1")
                tmpa, tmpb = W18("tmpa"), W18("tmpb")
                xi32 = wk.tile([qc, N * L], I32, name="xi32")
                idxf = wk.tile([128, 2 * N * L], F32, name="idxf")
                v.memset(idxf[:], 0.0)

                for l in range(L):
                    H_l, W_l = LV[l]
                    sl = slice(l * N, (l + 1) * N)
                    for (pt, src, scale) in ((xln, px, W_l / IMG_W), (yln, py, H_l / IMG_H)):
                        s.activation(pt[:, sl], src[:], ACTF.Copy, bias=-0.5, scale=scale)

                    for (xt, x0, w1, w0) in ((xln, x0t, wx1, wx0), (yln, y0t, wy1, wy0)):
                        v.tensor_copy(xi32[:, sl], xt[:, sl])
                        v.tensor_copy(tmpa[:, sl], xi32[:, sl])
                        v.tensor_tensor(out=tmpb[:, sl], in0=tmpa[:, sl], in1=xt[:, sl], op=ALU.is_gt)
                        v.tensor_tensor(out=x0[:, sl], in0=tmpa[:, sl], in1=tmpb[:, sl], op=ALU.subtract)
                        v.tensor_tensor(out=w1[:, sl], in0=xt[:, sl], in1=x0[:, sl], op=ALU.subtract)
                        s.activation(w0[:, sl], w1[:, sl], ACTF.Copy, bias=1.0, scale=-1.0)

                    for (vt, x0, lo, hi) in ((vx0, x0t, 0.0, W_l - 1.0), (vx1, x0t, -1.0, W_l - 2.0),
                                             (vy0, y0t, 0.0, H_l - 1.0), (vy1, y0t, -1.0, H_l - 2.0)):
                        v.tensor_scalar(tmpa[:, sl], x0[:, sl], lo, None, op0=ALU.is_ge)
                        v.tensor_scalar(tmpb[:, sl], x0[:, sl], hi, None, op0=ALU.is_le)
                        v.tensor_tensor(out=vt[:, sl], in0=tmpa[:, sl], in1=tmpb[:, sl], op=ALU.mult)

                    xs = tmpa[:, sl]
                    v.tensor_scalar(xs, x0t[:, sl], -1.0, W_l - 1.0, op0=ALU.max, op1=ALU.min)
                    ysA = tmpb[:, sl]
                    v.tensor_scalar(ysA, y0t[:, sl], 0.0, H_l - 1.0, op0=ALU.max, op1=ALU.min)
                    xsA = wk.tile([qc, N], F32, name="xsA")
                    v.tensor_scalar(xsA[:], xs, 1.0, None, op0=ALU.add)
                    # idx col layout: n*6 + l*2 + y
                    v.scalar_tensor_tensor(out=idxf[:qc, l * 2::2 * L], in0=ysA,
                                           scalar=float(W_l), in1=xsA[:],
                                           op0=ALU.mult, op1=ALU.add)
                    ysB = tmpb[:, sl]
                    v.tensor_scalar(ysB, y0t[:, sl], -1.0, H_l - 2.0, op0=ALU.max, op1=ALU.min)
                    v.tensor_scalar(xsA[:], xs, 1.0 + W_l, None, op0=ALU.add)
                    v.scalar_tensor_tensor(out=idxf[:qc, l * 2 + 1::2 * L], in0=ysB,
                                           scalar=float(W_l), in1=xsA[:],
                                           op0=ALU.mult, op1=ALU.add)

                for nm in ("wx1", "wy1", "wx0", "wy0", "vx0", "vx1", "vy0", "vy1"):
                    d[nm] = locals()[nm]
                d["idxf"], d["qc"], d["row0"] = idxf, qc, row0
                cd.append(d)

            # ---- phase B: wrap indices into dma_gather layout + gathers ----
            for ci in range(2):
                idxf = cd[ci]["idxf"]
                t1_ps = pT.tile([2 * NT, 128], F32, name="t1_ps", tag="tp", space="PSUM")
                nc.tensor.transpose(t1_ps[:], idxf[:], ident[:, :])
                t1s = wk.tile([2 * NT, 128], F32, name="t1s")
                s.copy(t1s[:], t1_ps[:])
                for ph in range(8):
                    t3_ps = pT.tile([16, 2 * NT], F32, name="t3_ps", tag="tp", space="PSUM")
                    nc.tensor.transpose(t3_ps[:], t1s[:, ph * 16:(ph + 1) * 16],
                                        ident[:2 * NT, :2 * NT])
                    # dest col for (n, l, y): (n*3+l)*32 + (ci*2+y)*8 + ph
                    wap = wallf[:, :]
                    dst = bass.AP(wap.tensor, wap.offset + ci * 16 + ph,
                                  [wap.ap[0], [96, N], [32, L], [8, 2]])
                    s.copy(dst, t3_ps[:])

            wall_ps = pW.tile([128, NT * WALL_S], F32, name="wall_ps", space="PSUM")
            nc.tensor.matmul(wall_ps[:, 0:512], lhsT=sel16[:], rhs=wallf[:, 0:512],
                             start=True, stop=True)
            nc.tensor.matmul(wall_ps[:, 512:NT * WALL_S], lhsT=sel16[:],
                             rhs=wallf[:, 512:NT * WALL_S], start=True, stop=True)
            v.tensor_copy(wall[:], wall_ps[:])

            g_tiles = {}
            for n in range(N):
                for l in range(L):
                    t = n * 3 + l
                    base = n * CAM_ROWS + LBASE[l]
                    src = bass.AP(featT.ap().tensor, base * C,
                                  [[C, ROWS_L[l] - 1], [1, 2 * C]])
                    g_t = gp.tile([128, 4 * 2 * C], F32, name=f"g_{n}_{l}", tag="g")
                    g3 = g_t[:].rearrange("p (i r) -> p i r", i=4)
                    nc.gpsimd.dma_gather(
                        out_ap=g3, in_ap=src,
                        idxs_ap=wall[:, t * WALL_S:(t + 1) * WALL_S],
                        num_idxs=NUM_IDX, num_idxs_reg=NUM_IDX,
                        elem_size=2 * C, elem_step=C,
                        queue_num=t % N_QUEUES,
                    )
                    g_tiles[(n, l)] = g_t

            # ---- phase C: attention weights + beta coefficients ----
            for ci, (row0, qc) in enumerate(CHUNKS):
                d = cd[ci]
                xc_t = wk.tile([qc, C], F32, name="xc_t")
                nc.sync.dma_start(xc_t[:], xq.ap()[row0:row0 + qc, :])
                xpc_t = wk.tile([qc, C], F32, name="xpc_t")
                nc.sync.dma_start(xpc_t[:], xqp.ap()[row0:row0 + qc, :])
                v.tensor_tensor(out=xc_t[:], in0=xc_t[:], in1=xpc_t[:], op=ALU.add)
                xT = []
                for k in range(2):
                    xT_ps = pT.tile([128, qc], F32, name="xT_ps", tag="tp", space="PSUM")
                    nc.tensor.transpose(xT_ps[:], xc_t[:, k * 128:(k + 1) * 128],
                                        ident[:qc, :qc])
                    xTk = wk.tile([128, qc], F32, name=f"xT{k}")
                    s.copy(xTk[:], xT_ps[:])
                    xT.append(xTk)
                at_ps = pT.tile([qc, 18], F32, name="at_ps", tag="tp", space="PSUM")
                nc.tensor.matmul(at_ps[:], lhsT=xT[0][:], rhs=wa_t[0][:], start=True, stop=False)
                nc.tensor.matmul(at_ps[:], lhsT=xT[1][:], rhs=wa_t[1][:], start=False, stop=False)
                nc.tensor.matmul(at_ps[:], lhsT=ones1[:1, :qc], rhs=ba_t[:], start=False, stop=True)
                w_t = wk.tile([qc, 18], F32, name="w_t")
                s.activation(w_t[:], at_ps[:], ACTF.Sigmoid)

                mask = wk.tile([qc, N], F32, name="mask")
                ta = wk.tile([qc, N], F32, name="ta")
                v.tensor_scalar(mask[:], d["z"], EPS, None, op0=ALU.is_gt)
                for (src_t, op, thr) in ((d["px"], ALU.is_gt, 0.0), (d["px"], ALU.is_lt, IMG_W),
                                         (d["py"], ALU.is_gt, 0.0), (d["py"], ALU.is_lt, IMG_H)):
                    v.tensor_scalar(ta[:], src_t[:], thr, None, op0=op)
                    v.tensor_tensor(out=mask[:], in0=mask[:], in1=ta[:], op=ALU.mult)

                beta = wk.tile([qc, 4 * N * L], F32, name="beta")
                pa = wk.tile([qc, N], F32, name="pa")
                pb = wk.tile([qc, N], F32, name="pb")
                u0 = wk.tile([qc, N], F32, name="u0")
                u1 = wk.tile([qc, N], F32, name="u1")
                al = wk.tile([qc, N], F32, name="al")
                for l in range(L):
                    sl = slice(l * N, (l + 1) * N)
                    v.tensor_tensor(out=al[:], in0=w_t[:, l::3], in1=mask[:], op=ALU.mult)
                    v.tensor_tensor(out=pa[:], in0=d["wy0"][:, sl], in1=d["vy0"][:, sl], op=ALU.mult)
                    v.tensor_tensor(out=pa[:], in0=pa[:], in1=al[:], op=ALU.mult)
                    v.tensor_tensor(out=pb[:], in0=d["wy1"][:, sl], in1=d["vy1"][:, sl], op=ALU.mult)
                    v.tensor_tensor(out=pb[:], in0=pb[:], in1=al[:], op=ALU.mult)
                    v.tensor_tensor(out=u0[:], in0=d["wx0"][:, sl], in1=d["vx0"][:, sl], op=ALU.mult)
                    v.tensor_tensor(out=u1[:], in0=d["wx1"][:, sl], in1=d["vx1"][:, sl], op=ALU.mult)
                    # beta col: n*12 + l*4 + y*2 + xc
                    st = 4 * L
                    v.tensor_tensor(out=beta[:, l * 4 + 0::st], in0=pa[:], in1=u0[:], op=ALU.mult)
                    v.tensor_tensor(out=beta[:, l * 4 + 1::st], in0=pa[:], in1=u1[:], op=ALU.mult)
                    v.tensor_tensor(out=beta[:, l * 4 + 2::st], in0=pb[:], in1=u0[:], op=ALU.mult)
                    v.tensor_tensor(out=beta[:, l * 4 + 3::st], in0=pb[:], in1=u1[:], op=ALU.mult)
                d["beta"] = beta

            # ---- phase D: weighted reduction over (cam, level, corner) ----
            # Interleave both chunks per gather table so G-pool slots free as
            # soon as a table's terms are consumed, keeping gathers flowing.
            is_pe = lambda t: t % PE_MOD != PE_MOD - 1
            n_pe = sum(1 for t in range(4 * N * L) if is_pe(t))
            accv, accp, pe_seen = [], [], [0, 0]
            for ci, (row0, qc) in enumerate(CHUNKS):
                av = wk.tile([qc, C], F32, name="accv")
                v.memset(av[:], 0.0)
                accv.append(av)
                accp.append(pA.tile([qc, C], F32, name="accp", tag="accp", space="PSUM"))
            for n in range(N):
                for l in range(L):
                    for ci, (row0, qc) in enumerate(CHUNKS):
                        beta = cd[ci]["beta"]
                        for y in range(2):
                            for xc in range(2):
                                t = n * 12 + l * 4 + y * 2 + xc
                                gsl = g_tiles[(n, l)][:qc, (ci * 2 + y) * 512 + xc * 256:
                                                     (ci * 2 + y) * 512 + (xc + 1) * 256]
                                bcol = beta[:, t:t + 1]
                                if is_pe(t):
                                    k = pe_seen[ci]; pe_seen[ci] += 1
                                    diag = dg.tile([128, 128], F32, name="diag", tag="diag")
                                    s.activation(diag[:qc, :qc], ident[:qc, :qc],
                                                 ACTF.Copy, scale=bcol)
                                    nc.tensor.matmul(accp[ci][:], lhsT=diag[:qc, :qc],
                                                     rhs=gsl, start=(k == 0),
                                                     stop=(k == n_pe - 1))
                                else:
                                    v.scalar_tensor_tensor(out=accv[ci][:], in0=gsl,
                                                           scalar=bcol, in1=accv[ci][:],
                                                           op0=ALU.mult, op1=ALU.add)
            for ci, (row0, qc) in enumerate(CHUNKS):
                fused = wk.tile([qc, C], F32, name="fused")
                v.tensor_tensor(out=fused[:], in0=accv[ci][:], in1=accp[ci][:], op=ALU.add)
                cd[ci]["fused"] = fused

            # ---- phase E: output projection + positional MLP ----
            for ci, (row0, qc) in enumerate(CHUNKS):
                d = cd[ci]
                out_ps = pO.tile([qc, C], F32, name="out_ps", tag="outp", space="PSUM")
                for k in range(2):
                    fT_ps = pT.tile([128, qc], F32, name="fT_ps", tag="tp", space="PSUM")
                    nc.tensor.transpose(fT_ps[:], d["fused"][:, k * 128:(k + 1) * 128],
                                        ident[:qc, :qc])
                    fTk = wk.tile([128, qc], F32, name=f"fT{k}")
                    s.copy(fTk[:], fT_ps[:])
                    nc.tensor.matmul(out_ps[:], lhsT=fTk[:], rhs=wo_t[k][:],
                                     start=(k == 0), stop=False)
                nc.tensor.matmul(out_ps[:], lhsT=ones1[:1, :qc], rhs=bo_t[:],
                                 start=False, stop=False)

                # inverse_sigmoid(ref) -> MLP
                refT = d["refT"]
                c01 = wk.tile([3, qc], F32, name="c01")
                v.tensor_scalar(c01[:], refT[:], 0.0, 1.0, op0=ALU.max, op1=ALU.min)
                x1 = wk.tile([3, qc], F32, name="x1")
                v.tensor_scalar(x1[:], c01[:], EPS, None, op0=ALU.max)
                x2 = wk.tile([3, qc], F32, name="x2")
                s.activation(x2[:], c01[:], ACTF.Copy, bias=1.0, scale=-1.0)
                v.tensor_scalar(x2[:], x2[:], EPS, None, op0=ALU.max)
                v.reciprocal(x2[:], x2[:])
                v.tensor_tensor(out=x1[:], in0=x1[:], in1=x2[:], op=ALU.mult)
                isgT = wk.tile([3, qc], F32, name="isgT")
                s.activation(isgT[:], x1[:], ACTF.Ln)

                h_ps = pH.tile([qc, C], F32, name="h_ps", tag="h", space="PSUM")
                nc.tensor.matmul(h_ps[:], lhsT=isgT[:], rhs=wp1_t[:], start=True, stop=False)
                nc.tensor.matmul(h_ps[:], lhsT=ones1[:1, :qc], rhs=bp1_t[:],
                                 start=False, stop=True)
                h_t = wk.tile([qc, C], F32, name="h_t")
                s.activation(h_t[:], h_ps[:], ACTF.Relu)
                for k in range(2):
                    hT_ps = pT.tile([128, qc], F32, name="hT_ps", tag="tp", space="PSUM")
                    nc.tensor.transpose(hT_ps[:], h_t[:, k * 128:(k + 1) * 128],
                                        ident[:qc, :qc])
                    hTk = wk.tile([128, qc], F32, name=f"hT{k}")
                    s.copy(hTk[:], hT_ps[:])
                    nc.tensor.matmul(out_ps[:], lhsT=hTk[:], rhs=wp2_t[k][:],
                                     start=False, stop=False)
                nc.tensor.matmul(out_ps[:], lhsT=ones1[:1, :qc], rhs=bp2_t[:],
                                 start=False, stop=True)

                out_sb = wk.tile([qc, C], F32, name="out_sb")
                v.tensor_copy(out_sb[:], out_ps[:])
                nc.sync.dma_start(out_d.ap()[row0:row0 + qc, :], out_sb[:])

    nc.compile()
    return nc


def _host_prep(inputs):
    """Build per-core input maps from full inputs."""
    feats = [inputs["feat0"], inputs["feat1"], inputs["feat2"]]
    lidar2img = np.asarray(inputs["lidar2img"], np.float32)

    featT_b = []
    for b in range(B):
        tbl = np.zeros((TOTAL_ROWS, C), np.float32)
        for n in range(N):
            for l in range(L):
                h, w = LV[l]
                base = n * CAM_ROWS + LBASE[l] + 1
                f = np.asarray(feats[l][b, n], np.float32)       # [C, H, W]
                tbl[base:base + h * w] = f.reshape(C, h * w).T
        featT_b.append(tbl)

    ident = np.eye(128, dtype=np.float32)
    sel16 = (np.arange(128)[None, :] % 16 == np.arange(16)[:, None]).astype(np.float32)

    in_maps = []
    for core in range(8):
        b, sidx = core // 4, core % 4
        rows = slice(sidx * NQ_CORE, (sidx + 1) * NQ_CORE)
        in_maps.append({
            "featT": featT_b[b],
            "xq": np.asarray(inputs["query"][b, rows], np.float32),
            "xqp": np.asarray(inputs["query_pos"][b, rows], np.float32),
            "refp": np.asarray(inputs["reference_points"][b, rows], np.float32),
            "l2iT": np.ascontiguousarray(lidar2img[b].transpose(2, 0, 1).reshape(4, 4 * N)),
            "Wa": np.asarray(inputs["W_attn"], np.float32),
            "ba": np.asarray(inputs["b_attn"], np.float32).reshape(1, 18),
            "Wo": np.asarray(inputs["W_out"], np.float32),
            "bo": np.asarray(inputs["b_out"], np.float32).reshape(1, C),
            "Wp1": np.asarray(inputs["W_pe1"], np.float32),
            "bp1": np.asarray(inputs["b_pe1"], np.float32).reshape(1, C),
            "Wp2": np.asarray(inputs["W_pe2"], np.float32),
            "bp2": np.asarray(inputs["b_pe2"], np.float32).reshape(1, C),
            "ident": ident,
            "sel16": sel16,
            "pcsb": np.array([[102.4, -51.2], [102.4, -51.2], [8.0, -5.0], [1.0, 0.0]],
                             np.float32),
        })
    return in_maps


def kernel(**inputs) -> np.ndarray:
    if "nc" not in _CACHE:
        _CACHE["nc"] = _build()
    nc = _CACHE["nc"]
    in_maps = _host_prep(inputs)
    res = run_bass_kernel_spmd(nc, in_maps, core_ids=list(range(8)),
                               **_CACHE.get("run_kwargs", {}))
    _CACHE["last_results"] = res
    out = np.zeros((B, Q, C), np.float32)
    for core in range(8):
        b, sidx = core // 4, core % 4
        out[b, sidx * NQ_CORE:(sidx + 1) * NQ_CORE] = res.results[core]["out"]
    return out
